# revision 17
# baseline (speedup 1.0000x reference)
"""Trainium2 Bass kernel for nn_ModelNVDP (neural-process with variational-dropout
per-sample weights), 8-core pure data parallel over batch B.

Key algebraic restructure: the per-sample weight tensors [B,out,in] are rank-1
sigmoid-gated, so every VDP einsum decomposes into shared-weight matmuls:
  mean = x@W.T - (s2*s3) * ((x*s1)@W.T) + (1-pb)*b
  var  = (s2*s3) * ((x*x*s1)@W2.T) - (s2*s3)^2 * ((x*s1)^2 @ W2.T) + pb(1-pb)b^2 + eps
with x*x*s1 = x*(x*s1) and (x*s1)^2, W2 = W*W.

Layout: activations feature-major [feature(partition), row(free)], rows blocked
per batch sample (256 rows). All weights pre-packed host-side to exact SBUF tile
layouts. Noise (reference threefry) generated host-side on CPU jax, pre-transposed
to feature-major, streamed per (b, layer).
"""

import math
from contextlib import ExitStack

import numpy as np

import concourse.bass as bass
import concourse.mybir as mybir
import concourse.tile as tile
from concourse import bacc
from concourse.bass_utils import run_bass_kernel_spmd

F32 = mybir.dt.float32
AF = mybir.ActivationFunctionType
ALU = mybir.AluOpType

# problem shapes (hardcoded per harness contract)
B, NC, NT = 256, 256, 256
XD, YD, RD, HD = 8, 4, 128, 512
IN1 = XD + RD            # 136
OUT4 = 2 * YD            # 8
ALPHA = -1.0
EPS = 1e-10
N_CORES = 8

# knobs
MM_DT = mybir.dt.float32r   # moving/stationary dtype for big matmuls
NZ_CHUNKS = 2               # split noise DMA per (b,layer) into this many DMAs

# per-net layout: (fin, fout, s1mt, s2mt, pbmt)  [mt = 128-row tiles in lanet out]
GCFG = {
    "g1": dict(fin=IN1, fout=HD, s1mt=2, s2mt=4, pbmt=4),
    "g2": dict(fin=HD, fout=HD, s1mt=4, s2mt=4, pbmt=4),
    "g3": dict(fin=HD, fout=HD, s1mt=4, s2mt=4, pbmt=4),
    "g4": dict(fin=HD, fout=OUT4, s1mt=4, s2mt=1, pbmt=1),
}
for _g, _c in GCFG.items():
    _c["la3mt"] = _c["s1mt"] + _c["s2mt"]       # mt index of the la3 row
    _c["pb0"] = _c["s1mt"] + _c["s2mt"] + 1     # first mt of the pb block
    _c["nmt"] = _c["pb0"] + _c["pbmt"]

LAN_HID = [RD, RD, RD, RD]   # lanet hidden dims (all 128)


# ---------------------------------------------------------------- host packing

def _np(x):
    return np.asarray(x, dtype=np.float32)


_noise_cache = {}


def _gen_noise():
    """Reference-matching threefry noise, feature-major [B, F, NT]."""
    if "nz" in _noise_cache:
        return _noise_cache["nz"]
    import jax
    cpu = jax.devices("cpu")[0]
    with jax.default_device(cpu):
        nk = jax.random.split(jax.random.key(42), 4)
        out = []
        for i in range(4):
            f = HD if i < 3 else OUT4
            n = np.asarray(jax.random.normal(nk[i], (B, NT, f), "float32"))
            out.append(np.ascontiguousarray(n.transpose(0, 2, 1)))  # [B, F, NT]
    _noise_cache["nz"] = out
    return out


def _pack_lanet_final(Wl, bl, cfg):
    """Reorder + zero-pad the final lanet layer so each gate block starts at an
    mt (128-row) boundary of the feature-major output. For g1 the s1 block is
    additionally permuted to the [r(128); x_t(8)] input order."""
    fin, fout = cfg["fin"], cfg["fout"]
    nmt = cfg["nmt"]
    Fp = nmt * 128
    W = np.zeros((Fp, RD), np.float32)
    b = np.zeros((Fp,), np.float32)

    # s1 block
    if fin == IN1:  # g1: permute to [8..135, 0..7]
        perm = np.concatenate([np.arange(8, IN1), np.arange(0, 8)])
    else:
        perm = np.arange(fin)
    W[:fin] = Wl[:fin][perm]
    b[:fin] = bl[:fin][perm]
    # s2 block
    o = cfg["s1mt"] * 128
    W[o:o + fout] = Wl[fin:fin + fout]
    b[o:o + fout] = bl[fin:fin + fout]
    # la3 row
    o = cfg["la3mt"] * 128
    W[o] = Wl[fin + fout]
    b[o] = bl[fin + fout]
    # pb block
    o = cfg["pb0"] * 128
    W[o:o + fout] = Wl[fin + fout + 1:]
    b[o:o + fout] = bl[fin + fout + 1:]
    return W, b


def _kt_pack(WT):
    """[fin, fout] -> [128, (fin//128) * fout] grouping k-tiles along free dim."""
    fin, fout = WT.shape
    kt = fin // 128
    return np.ascontiguousarray(
        WT.reshape(kt, 128, fout).transpose(1, 0, 2).reshape(128, kt * fout))


def _col_pack(v):
    """[fout] -> [128, fout//128] column layout (partition-major per o-tile)."""
    nt = v.shape[0] // 128
    return np.ascontiguousarray(v.reshape(nt, 128).T)


def pack_inputs(x_c, y_c, x_t, params, n_cores=N_CORES, bl=None):
    """Full inputs -> list of per-core input dicts (names match _build)."""
    x_c, y_c, x_t = _np(x_c), _np(y_c), _np(x_t)
    BL = bl if bl is not None else B // n_cores
    nz = _gen_noise()

    # ---- shared weight tensors (same on every core)
    shared = {}
    enc = params["d_enc"]
    # L0: [13, 512] with bias as 13th row
    W0 = _np(enc[0]["W"])   # [512, 12]
    shared["encW0T"] = np.concatenate([W0.T, _np(enc[0]["b"])[None, :]], 0)
    for l in range(1, 5):
        shared[f"encW{l}T"] = _kt_pack(_np(enc[l]["W"]).T)          # [128, 4*512]
        shared[f"encB{l}"] = _col_pack(_np(enc[l]["b"]))            # [128, 4]
    shared["encW5T"] = _kt_pack(_np(enc[5]["W"]).T / NC)            # [128, 4*128]
    shared["encB5"] = _np(enc[5]["b"]).reshape(128, 1)              # [128, 1]

    for g in ("g1", "g2", "g3", "g4"):
        cfg = GCFG[g]
        gp = params[g]
        W = _np(gp["W"])           # [fout, fin]
        bvec = _np(gp["b"])        # [fout]
        if g == "g1":
            # split into r-part (input features 8..135 -> kA) and x_t part (0..7 -> kB)
            shared["w1rT"] = np.ascontiguousarray(W[:, 8:].T)       # [128, 512]
            shared["w1r2T"] = np.ascontiguousarray((W[:, 8:] ** 2).T)
            shared["w1kT"] = np.ascontiguousarray(W[:, :8].T)       # [8, 512]
            shared["w1k2T"] = np.ascontiguousarray((W[:, :8] ** 2).T)
        else:
            shared[f"{g}WT"] = _kt_pack(W.T)
            shared[f"{g}W2T"] = _kt_pack((W ** 2).T)
        if cfg["fout"] >= 128:
            shared[f"{g}B"] = _col_pack(bvec)                        # [128, nt]
        else:
            bb = np.zeros((cfg["fout"], 1), np.float32)
            bb[:, 0] = bvec
            shared[f"{g}B"] = bb                                     # [8, 1]
        # lanet
        lan = gp["lanet"]
        for l in range(4):
            shared[f"lan{g}{l}T"] = np.ascontiguousarray(_np(lan[l]["W"]).T)  # [128,128]
            shared[f"lan{g}{l}B"] = _np(lan[l]["b"]).reshape(1, 128)
        W4, b4 = _pack_lanet_final(_np(lan[4]["W"]), _np(lan[4]["b"]), cfg)
        shared[f"lan{g}4T"] = np.ascontiguousarray(W4.T)             # [128, nmt*128]
        shared[f"lan{g}4B"] = b4.reshape(1, -1)                      # [1, nmt*128]

    in_maps = []
    for c in range(n_cores):
        bs = slice(c * BL, (c + 1) * BL)
        m = dict(shared)
        # encoder input [13, BL*256]: rows 0..7 x_c, 8..11 y_c, 12 ones
        ei = np.empty((13, BL * NC), np.float32)
        ei[0:8] = x_c[bs].transpose(2, 0, 1).reshape(8, -1)
        ei[8:12] = y_c[bs].transpose(2, 0, 1).reshape(4, -1)
        ei[12] = 1.0
        m["enc_in"] = ei
        m["onesrow"] = np.ones((1, 128), np.float32)
        m["xt_in"] = np.ascontiguousarray(
            x_t[bs].transpose(0, 2, 1))                              # [BL, 8, 256]
        for i in range(3):
            m[f"nz{i + 1}"] = np.ascontiguousarray(nz[i][bs])        # [BL, 512, 256]
        m["nz4"] = np.ascontiguousarray(nz[3][bs])                   # [BL, 8, 256]
        in_maps.append(m)
    return in_maps


def unpack_outputs(results, n_cores=N_CORES, bl=None):
    BL = bl if bl is not None else B // n_cores
    mus, sds = [], []
    for c in range(n_cores):
        mu = results[c]["muT"].reshape(4, BL, NT).transpose(1, 2, 0)
        sd = results[c]["sdT"].reshape(4, BL, NT).transpose(1, 2, 0)
        mus.append(mu)
        sds.append(sd)
    return np.concatenate(mus, 0), np.concatenate(sds, 0)


# ---------------------------------------------------------------- bass program

def _mm(x):
    return x


def _f32(x):
    return x.bitcast(F32)


def build_program(taus, BL=B // N_CORES, n_cores=N_CORES, repeat=1):
    """Build the per-core Bass program (same program for every core).
    repeat>1 re-emits the whole (idempotent) body for slope-based timing."""
    nc = bacc.Bacc("TRN2", target_bir_lowering=False, debug=False,
                   enable_asserts=False, num_devices=n_cores)

    dram = {}

    def din(name, shape, dt=F32):
        dram[name] = nc.dram_tensor(name, list(shape), dt, kind="ExternalInput").ap()
        return dram[name]

    def dout(name, shape):
        dram[name] = nc.dram_tensor(name, list(shape), F32, kind="ExternalOutput").ap()
        return dram[name]

    din("onesrow", (1, 128), MM_DT)
    din("enc_in", (13, BL * NC), MM_DT)
    din("xt_in", (BL, 8, NT), MM_DT)
    for i in range(3):
        din(f"nz{i + 1}", (BL, HD, NT))
    din("nz4", (BL, OUT4, NT))
    din("encW0T", (13, HD), MM_DT)
    for l in range(1, 5):
        din(f"encW{l}T", (128, 4 * HD), MM_DT)
        din(f"encB{l}", (128, 4))
    din("encW5T", (128, 4 * RD), MM_DT)
    din("encB5", (128, 1))
    din("w1rT", (128, HD), MM_DT); din("w1r2T", (128, HD), MM_DT)
    din("w1kT", (8, HD), MM_DT); din("w1k2T", (8, HD), MM_DT)
    for g in ("g2", "g3", "g4"):
        fout = GCFG[g]["fout"]
        din(f"{g}WT", (128, 4 * fout), MM_DT)
        din(f"{g}W2T", (128, 4 * fout), MM_DT)
    for g in ("g1", "g2", "g3", "g4"):
        cfg = GCFG[g]
        din(f"{g}B", (128 if cfg["fout"] >= 128 else cfg["fout"],
                      max(1, cfg["fout"] // 128)))
        for l in range(4):
            din(f"lan{g}{l}T", (128, 128), MM_DT)
            din(f"lan{g}{l}B", (1, 128), MM_DT)
        din(f"lan{g}4T", (128, cfg["nmt"] * 128), MM_DT)
        din(f"lan{g}4B", (1, cfg["nmt"] * 128), MM_DT)
    dout("muT", (4, BL * NT))
    dout("sdT", (4, BL * NT))

    with tile.TileContext(nc) as tc:
        for _ in range(repeat):
            with ExitStack() as ctx:
                _emit(ctx, tc, nc, dram, taus, BL)
    nc.compile()
    return nc


def _emit(ctx, tc, nc, dram, taus, BL):
    pp = ctx.enter_context(tc.tile_pool(name="persist", bufs=1))
    wp = ctx.enter_context(tc.tile_pool(name="weights", bufs=1))
    psum = ctx.enter_context(tc.tile_pool(name="psum", bufs=2, space="PSUM"))

    ones = pp.tile([1, 128], MM_DT, tag="ones")
    nc.sync.dma_start(ones[:], dram["onesrow"][:])
    r_fm = pp.tile([128, BL], MM_DT, tag="r_fm")

    def load_weights(pool, names):
        out = {}
        for name in names:
            ap = dram[name]
            t = pool.tile(list(ap.shape), ap.dtype, tag=name)
            nc.sync.dma_start(t[:], ap[:])
            out[name] = t
        return out

    # persistent: VDP weights + per-net base biases (used in phases B and C)
    vdp_names = ["w1rT", "w1r2T", "w1kT", "w1k2T"]
    for g in ("g2", "g3", "g4"):
        vdp_names += [f"{g}WT", f"{g}W2T"]
    vdp_names += [f"{g}B" for g in ("g1", "g2", "g3", "g4")]
    wt = load_weights(wp, vdp_names)

    # ================= Phase A: encoder =================
    with tc.tile_pool(name="encw", bufs=1) as ewp, \
         tc.tile_pool(name="enc", bufs=3) as ep:
        enc_names = ["encW0T", "encW5T", "encB5"] + \
            [f"encW{l}T" for l in range(1, 5)] + [f"encB{l}" for l in range(1, 5)]
        wt.update(load_weights(ewp, enc_names))
        n_blk = BL * NC // 512
        for blk in range(n_blk):
            xin = ep.tile([13, 512], MM_DT, tag="xin")
            nc.sync.dma_start(xin[:], dram["enc_in"][:, bass.ts(blk, 512)])
            # L0
            h = ep.tile([128, 4, 512], MM_DT, tag="h")
            for ot in range(4):
                ps = psum.tile([128, 512], F32, tag="pm")
                nc.tensor.matmul(ps[:], _mm(wt["encW0T"][:, bass.ts(ot, 128)]),
                                 _mm(xin[:]), start=True, stop=True)
                nc.scalar.activation(h[:, ot, :], ps[:], AF.Relu)
            # L1..L4
            for l in range(1, 5):
                hn = ep.tile([128, 4, 512], MM_DT, tag="h")
                wl = wt[f"encW{l}T"]
                for ot in range(4):
                    ps = psum.tile([128, 512], F32, tag="pm")
                    for kt in range(4):
                        nc.tensor.matmul(
                            ps[:], _mm(wl[:, kt * HD + ot * 128:kt * HD + ot * 128 + 128]),
                            _mm(h[:, kt, :]), start=(kt == 0), stop=(kt == 3))
                    nc.scalar.activation(hn[:, ot, :], ps[:], AF.Relu,
                                         bias=wt[f"encB{l}"][:, ot:ot + 1])
                h = hn
            # L5 (scaled by 1/NC host-side) + per-b mean reduce
            ps = psum.tile([128, 512], F32, tag="pm")
            w5 = wt["encW5T"]
            for kt in range(4):
                nc.tensor.matmul(ps[:], _mm(w5[:, bass.ts(kt, 128)]),
                                 _mm(h[:, kt, :]), start=(kt == 0), stop=(kt == 3))
            for half in range(2):
                b = blk * 2 + half
                red = ep.tile([128, 1], F32, tag="red")
                nc.vector.tensor_reduce(red[:], ps[:, bass.ts(half, 256)],
                                        axis=mybir.AxisListType.X, op=ALU.add)
                nc.vector.tensor_scalar_add(r_fm[:, b:b + 1], red[:],
                                            wt["encB5"][:, 0:1])

    # ================= Phase B: lanet + gates =================
    gates = {}
    with tc.tile_pool(name="lanw", bufs=1) as lwp, \
         tc.tile_pool(name="lan", bufs=2) as lp:
        lan_names = []
        for g in ("g1", "g2", "g3", "g4"):
            lan_names += [f"lan{g}{l}T" for l in range(5 - 1)]
            lan_names += [f"lan{g}{l}B" for l in range(5 - 1)]
            lan_names += [f"lan{g}4T", f"lan{g}4B"]
        wt.update(load_weights(lwp, lan_names))
        for g in ("g1", "g2", "g3", "g4"):
            cfg = GCFG[g]
            nmt = cfg["nmt"]
            tau_s = 1.0 / taus[g]
            act = r_fm
            for l in range(4):
                ps = psum.tile([128, BL], F32, tag="pm")
                nc.tensor.matmul(ps[:], _mm(wt[f"lan{g}{l}T"][:]), _mm(act[:]),
                                 start=True, stop=False)
                nc.tensor.matmul(ps[:], _mm(wt[f"lan{g}{l}B"][:]),
                                 _mm(ones[0:1, 0:BL]), start=False, stop=True)
                t1 = lp.tile([128, BL], F32, tag="lt1")
                nc.vector.tensor_scalar_mul(t1[:], ps[:], 0.1)
                an = lp.tile([128, BL], MM_DT, tag="lact")
                nc.vector.tensor_max(an[:], ps[:], t1[:])
                act = an
            # final layer -> la [128, nmt, BL] (one psum bank, nmt*BL <= 416)
            pla = psum.tile([128, nmt * BL], F32, tag="pm")
            for mt in range(nmt):
                sl = pla[:, bass.ts(mt, BL)]
                nc.tensor.matmul(sl, _mm(wt[f"lan{g}4T"][:, bass.ts(mt, 128)]),
                                 _mm(act[:]), start=True, stop=False)
                nc.tensor.matmul(sl, _mm(wt[f"lan{g}4B"][:, bass.ts(mt, 128)]),
                                 _mm(ones[0:1, 0:BL]), start=False, stop=True)
            clip = lp.tile([128, nmt, BL], F32, tag="lclip")
            nc.vector.tensor_scalar(clip[:, :, :],
                                    pla[:].rearrange("p (mt b) -> p mt b", mt=nmt),
                                    ALPHA, 2.5, ALU.max, ALU.min)
            sig = pp.tile([128, nmt, BL], F32, tag=f"sig_{g}")
            nc.scalar.activation(sig[:, :, :], clip[:, :, :], AF.Sigmoid, scale=tau_s)
            gates[g] = dict(sig=sig)

            nt = max(1, cfg["fout"] // 128)
            op = 128 if cfg["fout"] >= 128 else cfg["fout"]
            s2_0, la3mt, pb0 = cfg["s1mt"], cfg["la3mt"], cfg["pb0"]

            # s3 broadcast to all partitions via K=1 matmul
            s3row = lp.tile([1, BL], MM_DT, tag="ls3row")
            nc.vector.tensor_copy(s3row[:], sig[0:1, la3mt, :])
            ps3 = psum.tile([128, BL], F32, tag="pv")
            nc.tensor.matmul(ps3[:], _mm(ones[0:1, 0:128]),
                             _mm(s3row[:]), start=True, stop=True)
            c = pp.tile([op, nt, BL], F32, tag=f"c_{g}")
            ns = pp.tile([op, nt, BL], F32, tag=f"ns_{g}")
            cb = pp.tile([op, nt, BL], F32, tag=f"cb_{g}")
            bm = pp.tile([op, nt, BL], F32, tag=f"bm_{g}")
            for ot in range(nt):
                nc.vector.tensor_mul(c[:, ot, :], sig[0:op, s2_0 + ot, :],
                                     ps3[0:op, :])
            nc.vector.tensor_scalar_mul(ns[:, :, :], c[:, :, :], -1.0)
            bcol = wt[f"{g}B"]
            for ot in range(nt):
                pb = sig[0:op, pb0 + ot, :]
                q = lp.tile([op, BL], F32, tag="lq")
                nc.vector.tensor_scalar(q[:], pb, -1.0, 1.0, ALU.mult, ALU.add)
                t = lp.tile([op, BL], F32, tag="lt")
                nc.vector.tensor_mul(t[:], pb, q[:])
                b2 = lp.tile([op, 1], F32, tag="lb2")
                nc.vector.tensor_mul(b2[:], bcol[:, ot:ot + 1], bcol[:, ot:ot + 1])
                nc.vector.tensor_scalar(cb[:, ot, :], t[:], b2[:], EPS,
                                        ALU.mult, ALU.add)
                nc.vector.tensor_scalar_mul(bm[:, ot, :], q[:], bcol[:, ot:ot + 1])
            gates[g].update(c=c, ns=ns, cb=cb, bm=bm)

        # ---- g1 r-folding: fold the r-part of the g1 layer into per-(b,o) consts
        g1 = gates["g1"]
        sig1 = g1["sig"]
        rs1 = lp.tile([128, BL], MM_DT, tag="rs1")
        nc.vector.tensor_mul(rs1[:], r_fm[:], sig1[:, 0, :])
        r2s1 = lp.tile([128, BL], MM_DT, tag="r2s1")
        nc.vector.tensor_mul(r2s1[:], r_fm[:], rs1[:])
        rs1sq = lp.tile([128, BL], MM_DT, tag="rs1sq")
        nc.vector.tensor_mul(rs1sq[:], rs1[:], rs1[:])
        # one PSUM bank holds all 16 [128, BL] r-fold matvecs: [Y1R|Y2R|V1R|V2R] x 4 ot
        pfold = psum.tile([128, 16 * BL], F32, tag="pv")
        pyr = pfold[:, 0 * 4 * BL:1 * 4 * BL]
        pyr2 = pfold[:, 1 * 4 * BL:2 * 4 * BL]
        pvr = pfold[:, 2 * 4 * BL:3 * 4 * BL]
        pvr2 = pfold[:, 3 * 4 * BL:4 * 4 * BL]
        for ot in range(4):
            w1r = wt["w1rT"][:, bass.ts(ot, 128)]
            w1r2 = wt["w1r2T"][:, bass.ts(ot, 128)]
            nc.tensor.matmul(pyr[:, bass.ts(ot, BL)], _mm(w1r), _mm(r_fm[:]),
                             start=True, stop=True)
            nc.tensor.matmul(pyr2[:, bass.ts(ot, BL)], _mm(w1r), _mm(rs1[:]),
                             start=True, stop=True)
            nc.tensor.matmul(pvr[:, bass.ts(ot, BL)], _mm(w1r2), _mm(r2s1[:]),
                             start=True, stop=True)
            nc.tensor.matmul(pvr2[:, bass.ts(ot, BL)], _mm(w1r2), _mm(rs1sq[:]),
                             start=True, stop=True)
        for ot in range(4):
            nsot = g1["ns"][:, ot, :]
            cot = g1["c"][:, ot, :]
            # bias_mean = Y1R + ns*Y2R + bm
            t = lp.tile([128, BL], F32, tag="lfold")
            nc.vector.tensor_mul(t[:], nsot, pyr2[:, bass.ts(ot, BL)])
            t2 = lp.tile([128, BL], F32, tag="lfold2")
            nc.vector.tensor_add(t2[:], t[:], pyr[:, bass.ts(ot, BL)])
            nc.vector.tensor_add(g1["bm"][:, ot, :], t2[:], g1["bm"][:, ot, :])
            # cb' = cb + c*V1R - c^2*V2R
            u1 = lp.tile([128, BL], F32, tag="lfold3")
            nc.vector.tensor_mul(u1[:], cot, pvr[:, bass.ts(ot, BL)])
            u2 = lp.tile([128, BL], F32, tag="lfold4")
            nc.vector.tensor_mul(u2[:], cot, pvr2[:, bass.ts(ot, BL)])
            u3 = lp.tile([128, BL], F32, tag="lfold5")
            nc.vector.tensor_mul(u3[:], cot, u2[:])
            u4 = lp.tile([128, BL], F32, tag="lfold6")
            nc.vector.tensor_sub(u4[:], u1[:], u3[:])
            nc.vector.tensor_add(g1["cb"][:, ot, :], u4[:], g1["cb"][:, ot, :])

    # ================= Phase C: VDP chain =================
    with tc.tile_pool(name="vdp", bufs=2) as vp, \
         tc.tile_pool(name="epi", bufs=4) as sp:
        for b in range(BL):
            # ---- g1 (x_t part only; the r part is folded into bm/cb)
            hxs = vp.tile([8, 2, NT], MM_DT, tag="hxs1")
            nc.sync.dma_start(hxs[:, 0, :], dram["xt_in"][b])
            nc.scalar.activation(hxs[:, 1, :], hxs[:, 0, :], AF.Copy,
                                 scale=gates["g1"]["sig"][0:8, 1, b:b + 1])
            x2 = vp.tile([8, 2, NT], MM_DT, tag="x21")
            nc.gpsimd.tensor_mul(x2[:, 0, :], hxs[:, 0, :], hxs[:, 1, :])
            nc.gpsimd.tensor_mul(x2[:, 1, :], hxs[:, 1, :], hxs[:, 1, :])

            hxs, x2 = _vdp_layer(tc, nc, vp, sp, psum, dram, wt, gates, b,
                                 g="g1", kts=[("w1kT", "w1k2T", 8)],
                                 hxs=hxs, x2=x2, nz="nz1", nxt="g2", BL=BL)
            hxs, x2 = _vdp_layer(tc, nc, vp, sp, psum, dram, wt, gates, b,
                                 g="g2", kts=None, hxs=hxs, x2=x2, nz="nz2",
                                 nxt="g3", BL=BL)
            hxs, x2 = _vdp_layer(tc, nc, vp, sp, psum, dram, wt, gates, b,
                                 g="g3", kts=None, hxs=hxs, x2=x2, nz="nz3",
                                 nxt="g4", BL=BL)
            _vdp_layer(tc, nc, vp, sp, psum, dram, wt, gates, b,
                       g="g4", kts=None, hxs=hxs, x2=x2, nz="nz4",
                       nxt=None, BL=BL)


def _vdp_layer(tc, nc, vp, sp, psum, dram, wt, gates, b, g, kts, hxs, x2, nz,
               nxt, BL):
    cfg = GCFG[g]
    fout = cfg["fout"]
    nt = max(1, fout // 128)
    op = 128 if fout >= 128 else fout
    gt = gates[g]
    last = nxt is None

    # noise tile
    if fout >= 128:
        nzt = vp.tile([128, 4, NT], F32, tag="nzt")
        src = dram[nz][b].rearrange("(kt p) n -> p kt n", p=128)
        step = 4 // NZ_CHUNKS
        for ch in range(NZ_CHUNKS):
            nc.sync.dma_start(nzt[:, ch * step:(ch + 1) * step, :],
                              src[:, ch * step:(ch + 1) * step, :])
    else:
        nzt = vp.tile([op, 1, NT], F32, tag="nzt4")
        nc.sync.dma_start(nzt[:, 0, :], dram[nz][b])

    if not last:
        ncfg = GCFG[nxt]
        hxs_n = vp.tile([128, 4, 2 * NT], MM_DT, tag="hxsn")
        x2_n = vp.tile([128, 4, 2 * NT], MM_DT, tag="x2n")
        sig_n = gates[nxt]["sig"]

    for ot in range(nt):
        pm = psum.tile([op, 2 * NT], F32, tag="pm")
        pv = psum.tile([op, 2 * NT], F32, tag="pv")
        if kts is not None:  # g1: explicit (wname, w2name, ksize) list
            for j, (wn, w2n, ks) in enumerate(kts):
                st, fin_ = (j == 0), (j == len(kts) - 1)
                nc.tensor.matmul(pm[:], _mm(wt[wn][0:ks, bass.ts(ot, 128)]),
                                 _mm(hxs[0:ks, :, :]), start=st, stop=fin_)
                nc.tensor.matmul(pv[:], _mm(wt[w2n][0:ks, bass.ts(ot, 128)]),
                                 _mm(x2[0:ks, :, :]), start=st, stop=fin_)
        else:
            wto, w2to = wt[f"{g}WT"], wt[f"{g}W2T"]
            for kt in range(4):
                sl = slice(kt * fout + ot * 128, kt * fout + ot * 128 + op)
                nc.tensor.matmul(pm[:], _mm(wto[:, sl]), _mm(hxs[:, kt, :]),
                                 start=(kt == 0), stop=(kt == 3))
                nc.tensor.matmul(pv[:], _mm(w2to[:, sl]), _mm(x2[:, kt, :]),
                                 start=(kt == 0), stop=(kt == 3))

        ns_c = gt["ns"][:, ot, b:b + 1]
        # mean = y1 + ns*y2 + bm
        mt_ = sp.tile([op, NT], F32, tag="mt")
        nc.vector.tensor_scalar(mt_[:], pm[:, NT:2 * NT], ns_c,
                                gt["bm"][:, ot, b:b + 1], ALU.mult, ALU.add)
        mean = sp.tile([op, NT], F32, tag="mean")
        nc.vector.tensor_add(mean[:], mt_[:], pm[:, 0:NT])
        # w2s = v1 + ns*v2 ; std = sqrt(c*w2s + cb)
        wt_ = sp.tile([op, NT], F32, tag="wt")
        nc.vector.tensor_scalar_mul(wt_[:], pv[:, NT:2 * NT], ns_c)
        w2s = sp.tile([op, NT], F32, tag="w2s")
        nc.vector.tensor_add(w2s[:], wt_[:], pv[:, 0:NT])
        std = sp.tile([op, NT], F32, tag="std")
        nc.scalar.activation(std[:], w2s[:], AF.Sqrt,
                             scale=gt["c"][:, ot, b:b + 1],
                             bias=gt["cb"][:, ot, b:b + 1])
        sn = sp.tile([op, NT], F32, tag="sn")
        nc.gpsimd.tensor_mul(sn[:], std[:], nzt[:, ot if fout >= 128 else 0, :])
        outp = sp.tile([op, NT], F32, tag="outp")
        nc.vector.tensor_add(outp[:], sn[:], mean[:])

        if not last:
            nc.vector.tensor_scalar_max(hxs_n[:, ot, 0:NT], outp[:], 0.0)
            nc.scalar.activation(hxs_n[:, ot, NT:2 * NT], outp[:], AF.Relu,
                                 scale=sig_n[:, ot, b:b + 1])
            nc.gpsimd.tensor_mul(x2_n[:, ot, 0:NT], hxs_n[:, ot, 0:NT],
                                 hxs_n[:, ot, NT:2 * NT])
            nc.scalar.activation(x2_n[:, ot, NT:2 * NT], hxs_n[:, ot, NT:2 * NT],
                                 AF.Square)
        else:
            # engines need 0-aligned partition starts: run the softplus tail on
            # all 8 lanes (mu lanes produce harmless junk) and slice at the DMA.
            nc.gpsimd.dma_start(dram["muT"][:, b * NT:(b + 1) * NT], outp[0:4, :])
            e = sp.tile([8, NT], F32, tag="fe")
            nc.scalar.activation(e[:], outp[0:8, :], AF.Exp)
            spl = sp.tile([8, NT], F32, tag="fsp")
            nc.scalar.activation(spl[:], e[:], AF.Ln, bias=1.0)
            sd = sp.tile([8, NT], F32, tag="fsd")
            nc.vector.tensor_scalar(sd[:], spl[:], 0.9, 0.1, ALU.mult, ALU.add)
            nc.gpsimd.dma_start(dram["sdT"][:, b * NT:(b + 1) * NT], sd[4:8, :])

    if last:
        return None, None
    return hxs_n, x2_n


# ---------------------------------------------------------------- entry point

_prog_cache = {}


def _get_taus(params):
    out = {}
    for g in ("g1", "g2", "g3", "g4"):
        tau = float(np.clip(np.asarray(params[g]["tau"]).reshape(-1)[0], 0.5, 5.0))
        out[g] = tau
    return out


def kernel(x_c, y_c, x_t, params):
    taus = _get_taus(params)
    key = tuple(sorted(taus.items()))
    if key not in _prog_cache:
        _prog_cache[key] = build_program(taus)
    nc = _prog_cache[key]
    in_maps = pack_inputs(x_c, y_c, x_t, params)
    res = run_bass_kernel_spmd(nc, in_maps, core_ids=list(range(N_CORES)))
    return unpack_outputs(res.results)
